# revision 4
# baseline (speedup 1.0000x reference)
"""Trainium2 Bass kernel for nn_BOREP (dense_mlp):

    out[s, b, o] = einsum('sbi,oi->sbo', x, W) + bias[o]
    x [256, 64, 1024] f32, W [4096, 1024] f32, bias [4096] f32 -> out [256, 64, 4096] f32

Strategy
--------
Data-parallel over 8 NeuronCores: shard x along seq (axis 0), 32 timesteps per
core, i.e. per-core A = x-shard reshaped to [2048, 1024]; W replicated.
Per core: out_shard = A @ W.T -> [2048, 4096]; bias added on host (a free
numpy broadcast on the gathered result; b is identically zero here anyway).

Numeric scheme: single bf16 product. The rel-err tolerance (2e-2) is ~7x
looser than bf16's end-to-end rounding (~3e-3 incl. bf16 output), and on this
device bf16 matmul streams at 1 cycle/row -- the same per-instruction cost as
every other sub-fp32 dtype. HW microbenchmarks (see bench.py) showed that
fp8e4 DoubleRow runs at ~1.0 cyc/row here (2 k-tiles per instruction, i.e.
2 MAC/PE/cyc -- NOT the cost model's 0.5 cyc/row), so the previous session's
f32r+fp8 scheme (6144 PE-cyc/tile) and a 3-term all-fp8 scheme (also 6144 on
this silicon) both lose to one bf16 product:

    per [128m, 512n] out tile: 8 matmuls x 512 cyc = 4096 cyc
    128 tiles -> 524K cyc @ 2.4 GHz = 218.5 us/core, and the measured kernel
    sits at ~213-218 us -- PE-bound at ~100% utilization.

(Native fp32 would be 4x slower; f32r same speed but 2x the DMA bytes.)

Layout: host pre-blocks operands so every DMA lands [128, kt, free] tiles with
>=1KB-contiguous runs per partition; contraction dim k on SBUF partitions.
Loop is n-outer with the x-side SBUF-resident (16KB/partition) and W streamed
once (8MB total), double-buffered. Per-tile PSUM (6 banks deep) drains on the
DVE (copy + f32->bf16, ~0.6us vs 1.7us PE per tile); out-tile DMAs issue from
the otherwise-idle Pool (gpsimd) queue so the SP sequencer only carries input
DMAs. Total DMA 28MB/core (~85us at ~330GB/s), fully hidden under the PE.
Host epilogue: out = bf16.astype(f32) + bias.
"""
import sys

if "/opt/trn_rl_repo" not in sys.path:
    sys.path.insert(0, "/opt/trn_rl_repo")

import numpy as np
import ml_dtypes

# Problem constants (hardcoded per contest contract)
SEQ, BATCH, IN_DIM, OUT_DIM = 256, 64, 1024, 4096
N_CORES = 8
P = 128
K = IN_DIM
M = SEQ * BATCH // N_CORES     # 2048 rows per core
N = OUT_DIM
KT = K // P                    # 8 k-tiles
TM = 128                       # out-tile rows (PSUM partitions)
TN = 512                       # out-tile cols (one PSUM bank of fp32)
MT = M // TM                   # 16
NT = N // TN                   # 8

BF16 = ml_dtypes.bfloat16

_cache = {}


def _build_nc(repeat: int = 1):
    import concourse.mybir as mybir
    import concourse.tile as tile
    from concourse import bacc
    from contextlib import ExitStack

    F32 = mybir.dt.float32
    BF = mybir.dt.bfloat16

    nc = bacc.Bacc("TRN2", target_bir_lowering=False, debug=False)

    xb = nc.dram_tensor("xb", [MT, P, KT, TM], BF, kind="ExternalInput").ap()
    wb = nc.dram_tensor("wb", [NT, P, KT, TN], BF, kind="ExternalInput").ap()
    out = nc.dram_tensor("out", [M, N], BF, kind="ExternalOutput").ap()

    with tile.TileContext(nc) as tc:
        with ExitStack() as ctx:
            xpool = ctx.enter_context(tc.tile_pool(name="xpool", bufs=1))
            # bufs=3: with 2, the next iteration's first W slice waits on this
            # iteration's n=6 readers (~3.4us slack for a 1MB load); a third
            # buffer widens the slack to ~7us for 8KB/partition extra SBUF.
            wpool = ctx.enter_context(tc.tile_pool(name="wpool", bufs=3))
            opool = ctx.enter_context(tc.tile_pool(name="opool", bufs=8))
            ps = ctx.enter_context(tc.tile_pool(name="ps", bufs=6, space="PSUM"))

            for _ in range(repeat):
                # DMA emission order = consumption order: x[0] whole, then
                # wb[0] per k-tile (first matmul starts after one 128KB
                # chunk), then the rest of x; W n>=1 slices are emitted at
                # the top of the n-loop and prefetch one slice ahead
                # (bufs=2) on the SP queue, which carries no other traffic.
                x_sb = []
                t1 = xpool.tile([P, KT, TM], BF, tag="x_0")
                nc.sync.dma_start(t1[:], xb[0])
                x_sb.append(t1)
                w0 = wpool.tile([P, KT, TN], BF, tag="w")
                for k in range(KT):
                    nc.sync.dma_start(w0[:, k], wb[0, :, k])
                for m in range(1, MT):
                    t1 = xpool.tile([P, KT, TM], BF, tag=f"x_{m}")
                    nc.sync.dma_start(t1[:], xb[m])
                    x_sb.append(t1)

                for n in range(NT):
                    if n == 0:
                        wt = w0
                    else:
                        wt = wpool.tile([P, KT, TN], BF, tag="w")
                        nc.sync.dma_start(wt[:], wb[n])

                    for m in range(MT):
                        pm = ps.tile([P, TN], F32)
                        for k in range(KT):
                            nc.tensor.matmul(
                                pm[:], x_sb[m][:, k], wt[:, k],
                                start=(k == 0), stop=(k == KT - 1),
                            )
                        o_sb = opool.tile([P, TN], BF)
                        nc.vector.tensor_scalar_mul(o_sb[:], pm[:], 1.0)
                        nc.gpsimd.dma_start(
                            out[m * TM:(m + 1) * TM, n * TN:(n + 1) * TN],
                            o_sb[:],
                        )
    nc.compile()
    return nc


def get_nc():
    if "nc" not in _cache:
        _cache["nc"] = _build_nc()
    return _cache["nc"]


def _blk_x(a2d, dt):
    """[M, K] -> [MT, P, KT, TM] with blk[m, p, k, j] = a2d[m*TM+j, k*P+p]."""
    aT = np.ascontiguousarray(a2d.T)  # [K, M]
    return np.ascontiguousarray(
        aT.reshape(KT, P, MT, TM).transpose(2, 1, 0, 3)).astype(dt)


def _blk_w(wt, dt):
    """[K, N] -> [NT, P, KT, TN] with blk[n, p, k, j] = wt[k*P+p, n*TN+j]."""
    return np.ascontiguousarray(
        wt.reshape(KT, P, NT, TN).transpose(2, 1, 0, 3)).astype(dt)


def prep_in_maps(x, W, b):
    x = np.asarray(x, dtype=np.float32)
    W = np.asarray(W, dtype=np.float32)

    A = x.reshape(SEQ * BATCH, K)
    wblk = _blk_w(np.ascontiguousarray(W.T), BF16)

    in_maps = []
    for c in range(N_CORES):
        in_maps.append({
            "xb": _blk_x(A[c * M:(c + 1) * M], BF16),
            "wb": wblk,
        })
    return in_maps


def kernel(x, W, b):
    from concourse.bass_utils import run_bass_kernel_spmd

    in_maps = prep_in_maps(x, W, b)
    nc = get_nc()
    res = run_bass_kernel_spmd(nc, in_maps, core_ids=list(range(N_CORES)))
    full = np.concatenate([r["out"] for r in res.results], axis=0)
    out = full.astype(np.float32).reshape(SEQ, BATCH, OUT_DIM)
    out += np.asarray(b, dtype=np.float32)
    return out
